# revision 14
# baseline (speedup 1.0000x reference)
"""Multi-head causal attention (B=4, S=2048, D=1024, H=16) on 8 TRN2 NeuronCores.

Sharding: core c -> (batch c//2, head-group c%2 of 8 heads = 512 d_model cols).
Per core:
  - Q/K/V projections for its head slice (bf16 matmuls, fp32 accum).
    K/V biases are dropped on device: the K bias adds a per-query constant to
    every score, which cancels in softmax; the V bias contributes bv exactly
    (softmax weights sum to 1) and is folded into the host-side output bias
    b_eff = bv @ Wo + bo. Q bias kept (its per-key score term does not cancel).
  - causal attention, scores transposed ([keys, q]) so exp(scores)^T feeds the
    A@V matmul as the moving operand; V is augmented with a ones column so
    softmax sums fall out of the same matmul.
  - partial out-projection ctx^T @ Wo[rows-of-its-heads] (no bias)
Host: out[b] = partial[2b] + partial[2b+1] + b_eff.
"""

import numpy as np
import ml_dtypes
from contextlib import ExitStack

import concourse.bass as bass
import concourse.tile as tile
from concourse import bacc, mybir
from concourse.bass_utils import run_bass_kernel_spmd

F32 = mybir.dt.float32
BF16 = mybir.dt.bfloat16
EXP = mybir.ActivationFunctionType.Exp

N_CORES = 8
S = 2048          # sequence length
D = 1024          # d_model
HL = 8            # heads per core
HD = 64           # head dim
DL = HL * HD      # local d_model slice = 512
SCALE = 1.0 / 8.0  # 1/sqrt(HD)

NQT = S // 128    # 16 seq tiles of 128
NQC = S // 512    # 4 q chunks of 512
NDT = D // 128    # 8 d_model(in) tiles
NMT = DL // 128   # 4 local dout tiles (head pairs)

_compiled = None  # cached (nc,) so repeated kernel() calls skip rebuild


def _build():
    nc = bacc.Bacc("TRN2", target_bir_lowering=False, debug=False,
                   num_devices=N_CORES)

    xq_ap = nc.dram_tensor("xqt", [D, S], BF16, kind="ExternalInput").ap()
    xk_ap = nc.dram_tensor("xkt", [D, S], BF16, kind="ExternalInput").ap()
    xv_ap = nc.dram_tensor("xvt", [D, S], BF16, kind="ExternalInput").ap()
    wq_ap = nc.dram_tensor("wq", [D, DL], BF16, kind="ExternalInput").ap()
    wk_ap = nc.dram_tensor("wk", [D, DL], BF16, kind="ExternalInput").ap()
    wv_ap = nc.dram_tensor("wv", [D, DL], BF16, kind="ExternalInput").ap()
    bq_ap = nc.dram_tensor("bq", [DL, 1], F32, kind="ExternalInput").ap()
    wo_ap = nc.dram_tensor("wo", [DL, D], BF16, kind="ExternalInput").ap()
    out_ap = nc.dram_tensor("out", [S, D], F32, kind="ExternalOutput").ap()

    with tile.TileContext(nc) as tc, ExitStack() as ctx:
        wpool = ctx.enter_context(tc.tile_pool(name="weights", bufs=1))
        x_pool = ctx.enter_context(tc.tile_pool(name="x", bufs=7))
        qkv_pool = ctx.enter_context(tc.tile_pool(name="qkv", bufs=1))
        et_pool = ctx.enter_context(tc.tile_pool(name="et", bufs=6))
        norm_pool = ctx.enter_context(tc.tile_pool(name="norm", bufs=2))
        ot_pool = ctx.enter_context(tc.tile_pool(name="ot", bufs=2))
        ps_sc = ctx.enter_context(tc.tile_pool(name="ps_sc", bufs=2, space="PSUM"))
        ps_ctx = ctx.enter_context(tc.tile_pool(name="ps_ctx", bufs=2, space="PSUM"))
        ps_aux = ctx.enter_context(tc.tile_pool(name="ps_aux", bufs=2, space="PSUM"))

        # ---- consolidated weight loads (one DMA each), ordered so that the
        # first projection's inputs land first: wq, xq0, wk, xk0, wv, xv0 ----
        def load_w_flat(dram, nm, blk):
            # dram [NB*128, blk] -> sbuf [128, NB*blk], col nb*blk+j
            t = wpool.tile([128, (dram.shape[0] // 128) * blk], BF16, tag=nm,
                           name=nm)
            nb = dram.shape[0] // 128
            nc.sync.dma_start(
                t[:].rearrange("p (b j) -> p b j", b=nb),
                dram.rearrange("(b p) j -> p b j", p=128))
            return t

        def load_x_chunk(x_ap, qc, nm, eng=None):
            # x [D, 512 cols of chunk] -> [128, NDT*512], col d*512+s
            t = x_pool.tile([128, NDT * 512], BF16, tag="x", name=f"{nm}{qc}")
            (eng or nc.sync).dma_start(
                t[:].rearrange("p (d s) -> p d s", d=NDT),
                x_ap[:, 512 * qc:512 * (qc + 1)].rearrange(
                    "(d p) s -> p d s", p=128))
            return t

        # prologue DMA across all three DMA rings so transfers overlap:
        # sync carries weights, gpsimd carries xq/xk, scalar carries xv
        wq_sb = load_w_flat(wq_ap, "wq", DL)
        xq0 = load_x_chunk(xq_ap, 0, "q", nc.gpsimd)
        wk_sb = load_w_flat(wk_ap, "wk", DL)
        xk0 = load_x_chunk(xk_ap, 0, "k", nc.gpsimd)
        wv_sb = load_w_flat(wv_ap, "wv", DL)
        xv0 = load_x_chunk(xv_ap, 0, "v", nc.scalar)
        bq_sb = wpool.tile([128, NMT], F32, tag="bq")
        for m in range(NMT):
            nc.sync.dma_start(bq_sb[:, m:m + 1], bq_ap[128 * m:128 * (m + 1), :])
        wo_sb = [None]  # loaded late (first needed at chunk-1 attention)

        # persistent activations
        qT = [qkv_pool.tile([128, S], BF16, tag=f"qT{m}", name=f"qT{m}")
              for m in range(NMT)]
        kT = [qkv_pool.tile([128, S], BF16, tag=f"kT{m}", name=f"kT{m}")
              for m in range(NMT)]
        v_aug = [None] * NQT
        ctxT = [qkv_pool.tile([128, S], BF16, tag=f"ctxT{m}", name=f"ctxT{m}")
                for m in range(NMT)]

        # ---- projection groups (PE fillers) ----
        def proj_q(x_sb, qc, m):
            ps = ps_aux.tile([128, 512], F32, tag="aux", name="psq")
            for d in range(NDT):
                nc.tensor.matmul(
                    ps[:], wq_sb[:, 512 * d + 128 * m:512 * d + 128 * (m + 1)],
                    x_sb[:, 512 * d:512 * (d + 1)],
                    start=(d == 0), stop=(d == NDT - 1))
            nc.vector.tensor_scalar_add(
                qT[m][:, 512 * qc:512 * (qc + 1)], ps[:], bq_sb[:, m:m + 1])

        def proj_k(x_sb, qc, m):
            ps = ps_aux.tile([128, 512], F32, tag="aux", name="psk")
            for d in range(NDT):
                nc.tensor.matmul(
                    ps[:], wk_sb[:, 512 * d + 128 * m:512 * d + 128 * (m + 1)],
                    x_sb[:, 512 * d:512 * (d + 1)],
                    start=(d == 0), stop=(d == NDT - 1))
            nc.vector.tensor_copy(kT[m][:, 512 * qc:512 * (qc + 1)], ps[:])

        def proj_v(x_sb, qc, sti):
            st = 4 * qc + sti
            va = qkv_pool.tile([128, HL * (HD + 1)], BF16, tag=f"va{st}",
                               name=f"va{st}")
            nc.vector.memset(va[:], 1.0)
            ps = ps_aux.tile([128, 512], F32, tag="aux", name="psv")
            for d in range(NDT):
                nc.tensor.matmul(
                    ps[:], x_sb[:, 512 * d + 128 * sti:512 * d + 128 * (sti + 1)],
                    wv_sb[:, 512 * d:512 * (d + 1)],
                    start=(d == 0), stop=(d == NDT - 1))
            nc.vector.tensor_copy(
                va[:].rearrange("p (h c) -> p h c", h=HL)[:, :, 0:HD],
                ps[:].rearrange("p (h c) -> p h c", h=HL))
            v_aug[st] = va

        def emit_outproj(qt):
            pa = ps_aux.tile([128, 512], F32, tag="aux", name="pa")
            pb = ps_aux.tile([128, 512], F32, tag="aux", name="pb")
            for dm in range(NMT):
                st = ctxT[dm][:, 128 * qt:128 * (qt + 1)]
                nc.tensor.matmul(pa[:], st, wo_sb[0][:, 1024 * dm:1024 * dm + 512],
                                 start=(dm == 0), stop=(dm == NMT - 1))
                nc.tensor.matmul(pb[:], st,
                                 wo_sb[0][:, 1024 * dm + 512:1024 * dm + 1024],
                                 start=(dm == 0), stop=(dm == NMT - 1))
            ot = ot_pool.tile([128, 1024], F32, tag="ot", name="ot")
            nc.vector.tensor_copy(ot[:, 0:512], pa[:])
            nc.vector.tensor_copy(ot[:, 512:1024], pb[:])
            nc.sync.dma_start(out_ap[128 * qt:128 * (qt + 1), :], ot[:])

        def make_fillers(qc):
            """Immediate groups for chunk qc (run during chunk qc-1: v proj +
            pair-0 q/k) and deferred groups (pair 1-3 q/k, run inside chunk qc
            while earlier pairs compute)."""
            xq_c = load_x_chunk(xq_ap, qc, "q", nc.gpsimd if qc == 1 else None)
            xk_c = load_x_chunk(xk_ap, qc, "k", nc.gpsimd if qc == 1 else None)
            xv_c = load_x_chunk(xv_ap, qc, "v", nc.scalar if qc == 1 else None)
            imm = [lambda: proj_q(xq_c, qc, 0), lambda: proj_k(xk_c, qc, 0)]
            imm += [lambda s=s: proj_v(xv_c, qc, s) for s in range(4)]
            dfr = []
            for m in range(1, NMT):
                dfr.append(lambda m=m: proj_q(xq_c, qc, m))
                dfr.append(lambda m=m: proj_k(xk_c, qc, m))
            return imm, dfr

        # ---- chunk-0 pair-0 q/k + v inline (DMA-paced prologue) ----
        proj_q(xq0, 0, 0)
        proj_k(xk0, 0, 0)
        for s in range(4):
            proj_v(xv0, 0, s)
        dfr_cur = []
        for m in range(1, NMT):
            dfr_cur.append(lambda m=m: proj_q(xq0, 0, m))
            dfr_cur.append(lambda m=m: proj_k(xk0, 0, m))

        # ---- attention ----
        for qc in range(NQC):
            if qc + 1 < NQC:
                imm_next, dfr_next = make_fillers(qc + 1)
            else:
                imm_next, dfr_next = [], []
            if qc == 0:
                # wo arrives behind all x-chunk DMAs it would otherwise delay
                wo_sb[0] = load_w_flat(wo_ap, "wo", D)
            ops = ([lambda qt=4 * (qc - 1) + j: emit_outproj(qt)
                    for j in range(4)] if qc > 0 else [])
            rest = []
            for i, g in enumerate(imm_next):
                rest.append(g)
                if i % 2 == 1 and ops:
                    rest.append(ops.pop(0))
            rest += ops
            # deferred q/k proj of THIS chunk pinned to segment starts so
            # (q_m, k_m) complete during pair m-1
            if not imm_next:
                # last chunk: deferred first, outprojs late (feed pair 3)
                fillers = dfr_cur + rest
            else:
                total = len(dfr_cur) + len(rest)
                fillers = []
                ri = 0
                for seg in range(4):
                    if seg < 3 and 2 * seg + 1 < len(dfr_cur):
                        fillers += [dfr_cur[2 * seg], dfr_cur[2 * seg + 1]]
                    tgt = ((seg + 1) * total + 3) // 4
                    while len(fillers) < tgt and ri < len(rest):
                        fillers.append(rest[ri])
                        ri += 1
                fillers += rest[ri:]
            dfr_cur = dfr_next
            nf = 0
            nkt = 4 * (qc + 1)  # causal: key tiles 0..nkt-1
            for hp in range(HL // 2):
                m = hp
                heads = (2 * hp, 2 * hp + 1)
                ctx_ps = {h: ps_ctx.tile([HD + 1, 512], F32, tag="ctx",
                                         name=f"ctx{h}") for h in heads}

                def emit_scores_exp(kt):
                    qs = max(0, 128 * kt - 512 * qc)  # local q start
                    sc = ps_sc.tile([128, 1024], F32, tag="sc", name="sc")
                    for i, h in enumerate(heads):
                        po = 64 * i
                        nc.tensor.matmul(
                            sc[:, 512 * i + qs:512 * (i + 1)],
                            kT[m][po:po + HD, 128 * kt:128 * (kt + 1)],
                            qT[m][po:po + HD, 512 * qc + qs:512 * (qc + 1)],
                            start=True, stop=True)
                    et = et_pool.tile([128, 1024], BF16, tag="et", name="et")
                    nc.scalar.activation(et[:, qs:1024], sc[:, qs:1024],
                                         EXP, scale=SCALE)
                    if 4 * qc <= kt < 4 * qc + 4:  # diagonal: mask k>q
                        for i in range(2):
                            nc.gpsimd.affine_select(
                                out=et[:, 512 * i + qs:512 * i + qs + 128],
                                in_=et[:, 512 * i + qs:512 * i + qs + 128],
                                compare_op=mybir.AluOpType.is_ge, fill=0.0,
                                base=0, pattern=[[1, 128]],
                                channel_multiplier=-1)
                    return et

                def emit_ctx(kt, et):
                    qs = max(0, 128 * kt - 512 * qc)
                    for i, h in enumerate(heads):
                        nc.tensor.matmul(
                            ctx_ps[h][:, qs:512],
                            v_aug[kt][:].rearrange(
                                "p (h c) -> p h c", h=HL)[:, h, :],
                            et[:, 512 * i + qs:512 * (i + 1)],
                            start=(kt == 0), stop=(kt == nkt - 1))

                pend = []
                for kt in range(nkt):
                    pend.append((kt, emit_scores_exp(kt)))
                    if len(pend) > 4:
                        emit_ctx(*pend.pop(0))
                    want = (len(fillers) * (hp * nkt + kt + 1)) // (HL // 2 * nkt)
                    while nf < want:
                        fillers[nf]()
                        nf += 1
                for p in pend:
                    emit_ctx(*p)

                # normalize: one cu tile per pair, batched reciprocal
                cu = norm_pool.tile([128, 1024], F32, tag="cu", name="cu")
                sums_sb = norm_pool.tile([1, 1024], F32, tag="sums", name="sums")
                for i, h in enumerate(heads):
                    nc.vector.tensor_copy(cu[0:HD, 512 * i:512 * (i + 1)],
                                          ctx_ps[h][0:HD, :])
                    nc.vector.tensor_copy(sums_sb[:, 512 * i:512 * (i + 1)],
                                          ctx_ps[h][HD:HD + 1, :])
                recip = norm_pool.tile([1, 1024], F32, tag="recip", name="recip")
                nc.vector.reciprocal_approx_fast(recip[:], sums_sb[:])
                rep = norm_pool.tile([HD, 1024], F32, tag="rep", name="rep")
                for i in range(2):
                    nc.gpsimd.partition_broadcast(
                        rep[:, 512 * i:512 * (i + 1)],
                        recip[:, 512 * i:512 * (i + 1)])
                for i, h in enumerate(heads):
                    po = 64 * i
                    nc.vector.tensor_mul(
                        ctxT[m][po:po + HD, 512 * qc:512 * (qc + 1)],
                        cu[0:HD, 512 * i:512 * (i + 1)],
                        rep[:, 512 * i:512 * (i + 1)])
            while nf < len(fillers):
                fillers[nf]()
                nf += 1

        for qt in range(4 * (NQC - 1), 4 * NQC):
            emit_outproj(qt)

    nc.compile()
    return nc


def _shard(inputs):
    xt = {}
    for nm in ("inputs_q", "inputs_k", "inputs_v"):
        xt[nm] = [np.ascontiguousarray(inputs[nm][b].T.astype(ml_dtypes.bfloat16))
                  for b in range(4)]
    w16 = {nm: inputs[nm].astype(ml_dtypes.bfloat16)
           for nm in ("Wq", "Wk", "Wv", "Wo")}
    in_maps = []
    for c in range(N_CORES):
        b, g = c // 2, c % 2
        sl = slice(512 * g, 512 * (g + 1))
        in_maps.append({
            "xqt": xt["inputs_q"][b],
            "xkt": xt["inputs_k"][b],
            "xvt": xt["inputs_v"][b],
            "wq": np.ascontiguousarray(w16["Wq"][:, sl]),
            "wk": np.ascontiguousarray(w16["Wk"][:, sl]),
            "wv": np.ascontiguousarray(w16["Wv"][:, sl]),
            "bq": np.ascontiguousarray(inputs["bq"][sl])[:, None],
            "wo": np.ascontiguousarray(w16["Wo"][sl, :]),
        })
    return in_maps


def kernel(**inputs):
    global _compiled
    inputs = {k: np.asarray(v, dtype=np.float32) for k, v in inputs.items()}
    if _compiled is None:
        _compiled = _build()
    nc = _compiled
    in_maps = _shard(inputs)
    res = run_bass_kernel_spmd(nc, in_maps, list(range(N_CORES)),
                               trace=bool(int(__import__("os").environ.get("BASS_TRACE", "0"))))
    kernel.last_results = res
    B = 4
    b_eff = inputs["bv"] @ inputs["Wo"] + inputs["bo"]
    out = np.empty((B, S, D), np.float32)
    for b in range(B):
        out[b] = res.results[2 * b]["out"] + res.results[2 * b + 1]["out"]
    out += b_eff[None, None, :]
    return out


# revision 16
# speedup vs baseline: 1.0219x; 1.0219x over previous
"""Multi-head causal attention (B=4, S=2048, D=1024, H=16) on 8 TRN2 NeuronCores.

Sharding: core c -> (batch c//2, head-group c%2 of 8 heads = 512 d_model cols).
Per core:
  - Q/K/V projections for its head slice (bf16 matmuls, fp32 accum).
    K/V biases are dropped on device: the K bias adds a per-query constant to
    every score, which cancels in softmax; the V bias contributes bv exactly
    (softmax weights sum to 1) and is folded into the host-side output bias
    b_eff = bv @ Wo + bo. Q bias kept (its per-key score term does not cancel).
  - causal attention, scores transposed ([keys, q]) so exp(scores)^T feeds the
    A@V matmul as the moving operand; V is augmented with a ones column so
    softmax sums fall out of the same matmul.
  - partial out-projection ctx^T @ Wo[rows-of-its-heads] (no bias)
Host: out[b] = partial[2b] + partial[2b+1] + b_eff.
"""

import numpy as np
import ml_dtypes
from contextlib import ExitStack

import concourse.bass as bass
import concourse.tile as tile
from concourse import bacc, mybir
from concourse.bass_utils import run_bass_kernel_spmd

F32 = mybir.dt.float32
BF16 = mybir.dt.bfloat16
EXP = mybir.ActivationFunctionType.Exp

N_CORES = 8
S = 2048          # sequence length
D = 1024          # d_model
HL = 8            # heads per core
HD = 64           # head dim
DL = HL * HD      # local d_model slice = 512
SCALE = 1.0 / 8.0  # 1/sqrt(HD)

NQT = S // 128    # 16 seq tiles of 128
NQC = S // 512    # 4 q chunks of 512
NDT = D // 128    # 8 d_model(in) tiles
NMT = DL // 128   # 4 local dout tiles (head pairs)

_compiled = None  # cached (nc,) so repeated kernel() calls skip rebuild


def _build():
    nc = bacc.Bacc("TRN2", target_bir_lowering=False, debug=False,
                   num_devices=N_CORES)

    xq_ap = nc.dram_tensor("xqt", [D, S], BF16, kind="ExternalInput").ap()
    xk_ap = nc.dram_tensor("xkt", [D, S], BF16, kind="ExternalInput").ap()
    xv_ap = nc.dram_tensor("xvt", [D, S], BF16, kind="ExternalInput").ap()
    wq_ap = nc.dram_tensor("wq", [D, DL], BF16, kind="ExternalInput").ap()
    wk_ap = nc.dram_tensor("wk", [D, DL], BF16, kind="ExternalInput").ap()
    wv_ap = nc.dram_tensor("wv", [D, DL], BF16, kind="ExternalInput").ap()
    bq_ap = nc.dram_tensor("bq", [DL, 1], F32, kind="ExternalInput").ap()
    wo_ap = nc.dram_tensor("wo", [DL, D], BF16, kind="ExternalInput").ap()
    out_ap = nc.dram_tensor("out", [S, D], F32, kind="ExternalOutput").ap()

    with tile.TileContext(nc) as tc, ExitStack() as ctx:
        wpool = ctx.enter_context(tc.tile_pool(name="weights", bufs=1))
        x_pool = ctx.enter_context(tc.tile_pool(name="x", bufs=7))
        qkv_pool = ctx.enter_context(tc.tile_pool(name="qkv", bufs=1))
        et_pool = ctx.enter_context(tc.tile_pool(name="et", bufs=6))
        norm_pool = ctx.enter_context(tc.tile_pool(name="norm", bufs=2))
        ot_pool = ctx.enter_context(tc.tile_pool(name="ot", bufs=2))
        ps_sc = ctx.enter_context(tc.tile_pool(name="ps_sc", bufs=2, space="PSUM"))
        ps_ctx = ctx.enter_context(tc.tile_pool(name="ps_ctx", bufs=2, space="PSUM"))
        ps_aux = ctx.enter_context(tc.tile_pool(name="ps_aux", bufs=2, space="PSUM"))

        # ---- consolidated weight loads (one DMA each), ordered so that the
        # first projection's inputs land first: wq, xq0, wk, xk0, wv, xv0 ----
        def load_w_flat(dram, nm, blk):
            # dram [NB*128, blk] -> sbuf [128, NB*blk], col nb*blk+j
            t = wpool.tile([128, (dram.shape[0] // 128) * blk], BF16, tag=nm,
                           name=nm)
            nb = dram.shape[0] // 128
            nc.sync.dma_start(
                t[:].rearrange("p (b j) -> p b j", b=nb),
                dram.rearrange("(b p) j -> p b j", p=128))
            return t

        def load_x_chunk(x_ap, qc, nm, eng=None):
            # x [D, 512 cols of chunk] -> [128, NDT*512], col d*512+s
            t = x_pool.tile([128, NDT * 512], BF16, tag="x", name=f"{nm}{qc}")
            (eng or nc.sync).dma_start(
                t[:].rearrange("p (d s) -> p d s", d=NDT),
                x_ap[:, 512 * qc:512 * (qc + 1)].rearrange(
                    "(d p) s -> p d s", p=128))
            return t

        # prologue DMA: q/k path serial on the sync ring (priority order);
        # v path on the scalar ring so it overlaps instead of trailing
        wq_sb = load_w_flat(wq_ap, "wq", DL)
        xq0 = load_x_chunk(xq_ap, 0, "q")
        wk_sb = load_w_flat(wk_ap, "wk", DL)
        xk0 = load_x_chunk(xk_ap, 0, "k")
        wv_sb = wpool.tile([128, NDT * DL], BF16, tag="wv", name="wv")
        nc.scalar.dma_start(
            wv_sb[:].rearrange("p (b j) -> p b j", b=NDT),
            wv_ap.rearrange("(b p) j -> p b j", p=128))
        xv0 = load_x_chunk(xv_ap, 0, "v", nc.scalar)
        bq_sb = wpool.tile([128, NMT], F32, tag="bq")
        for m in range(NMT):
            nc.sync.dma_start(bq_sb[:, m:m + 1], bq_ap[128 * m:128 * (m + 1), :])
        wo_sb = [None]  # loaded late (first needed at chunk-1 attention)

        # persistent activations
        qT = [qkv_pool.tile([128, S], BF16, tag=f"qT{m}", name=f"qT{m}")
              for m in range(NMT)]
        kT = [qkv_pool.tile([128, S], BF16, tag=f"kT{m}", name=f"kT{m}")
              for m in range(NMT)]
        v_aug = [None] * NQT
        ctxT = [qkv_pool.tile([128, S], BF16, tag=f"ctxT{m}", name=f"ctxT{m}")
                for m in range(NMT)]

        # ---- projection groups (PE fillers) ----
        def proj_q(x_sb, qc, m):
            ps = ps_aux.tile([128, 512], F32, tag="aux", name="psq")
            for d in range(NDT):
                nc.tensor.matmul(
                    ps[:], wq_sb[:, 512 * d + 128 * m:512 * d + 128 * (m + 1)],
                    x_sb[:, 512 * d:512 * (d + 1)],
                    start=(d == 0), stop=(d == NDT - 1))
            nc.vector.tensor_scalar_add(
                qT[m][:, 512 * qc:512 * (qc + 1)], ps[:], bq_sb[:, m:m + 1])

        def proj_k(x_sb, qc, m):
            ps = ps_aux.tile([128, 512], F32, tag="aux", name="psk")
            for d in range(NDT):
                nc.tensor.matmul(
                    ps[:], wk_sb[:, 512 * d + 128 * m:512 * d + 128 * (m + 1)],
                    x_sb[:, 512 * d:512 * (d + 1)],
                    start=(d == 0), stop=(d == NDT - 1))
            nc.vector.tensor_copy(kT[m][:, 512 * qc:512 * (qc + 1)], ps[:])

        def proj_v(x_sb, qc, sti):
            st = 4 * qc + sti
            va = qkv_pool.tile([128, HL * (HD + 1)], BF16, tag=f"va{st}",
                               name=f"va{st}")
            nc.vector.memset(va[:], 1.0)
            ps = ps_aux.tile([128, 512], F32, tag="aux", name="psv")
            for d in range(NDT):
                nc.tensor.matmul(
                    ps[:], x_sb[:, 512 * d + 128 * sti:512 * d + 128 * (sti + 1)],
                    wv_sb[:, 512 * d:512 * (d + 1)],
                    start=(d == 0), stop=(d == NDT - 1))
            nc.vector.tensor_copy(
                va[:].rearrange("p (h c) -> p h c", h=HL)[:, :, 0:HD],
                ps[:].rearrange("p (h c) -> p h c", h=HL))
            v_aug[st] = va

        def emit_outproj(qt):
            pa = ps_aux.tile([128, 512], F32, tag="aux", name="pa")
            pb = ps_aux.tile([128, 512], F32, tag="aux", name="pb")
            for dm in range(NMT):
                st = ctxT[dm][:, 128 * qt:128 * (qt + 1)]
                nc.tensor.matmul(pa[:], st, wo_sb[0][:, 1024 * dm:1024 * dm + 512],
                                 start=(dm == 0), stop=(dm == NMT - 1))
                nc.tensor.matmul(pb[:], st,
                                 wo_sb[0][:, 1024 * dm + 512:1024 * dm + 1024],
                                 start=(dm == 0), stop=(dm == NMT - 1))
            ot = ot_pool.tile([128, 1024], F32, tag="ot", name="ot")
            nc.vector.tensor_copy(ot[:, 0:512], pa[:])
            nc.vector.tensor_copy(ot[:, 512:1024], pb[:])
            nc.sync.dma_start(out_ap[128 * qt:128 * (qt + 1), :], ot[:])

        def make_fillers(qc):
            """Immediate groups for chunk qc (run during chunk qc-1: v proj +
            pair-0 q/k) and deferred groups (pair 1-3 q/k, run inside chunk qc
            while earlier pairs compute)."""
            xq_c = load_x_chunk(xq_ap, qc, "q")
            xk_c = load_x_chunk(xk_ap, qc, "k")
            xv_c = load_x_chunk(xv_ap, qc, "v", nc.scalar if qc == 1 else None)
            imm = [lambda: proj_q(xq_c, qc, 0), lambda: proj_k(xk_c, qc, 0)]
            imm += [lambda s=s: proj_v(xv_c, qc, s) for s in range(4)]
            dfr = []
            for m in range(1, NMT):
                dfr.append(lambda m=m: proj_q(xq_c, qc, m))
                dfr.append(lambda m=m: proj_k(xk_c, qc, m))
            return imm, dfr

        # ---- chunk-0 pair-0 q/k + v inline (DMA-paced prologue) ----
        proj_q(xq0, 0, 0)
        proj_k(xk0, 0, 0)
        for s in range(4):
            proj_v(xv0, 0, s)
        dfr_cur = []
        for m in range(1, NMT):
            dfr_cur.append(lambda m=m: proj_q(xq0, 0, m))
            dfr_cur.append(lambda m=m: proj_k(xk0, 0, m))

        # ---- attention ----
        for qc in range(NQC):
            if qc + 1 < NQC:
                imm_next, dfr_next = make_fillers(qc + 1)
            else:
                imm_next, dfr_next = [], []
            if qc == 0:
                # wo arrives behind all x-chunk DMAs it would otherwise delay
                wo_sb[0] = load_w_flat(wo_ap, "wo", D)
            ops = ([lambda qt=4 * (qc - 1) + j: emit_outproj(qt)
                    for j in range(4)] if qc > 0 else [])
            rest = []
            for i, g in enumerate(imm_next):
                rest.append(g)
                if i % 2 == 1 and ops:
                    rest.append(ops.pop(0))
            rest += ops
            # deferred q/k proj of THIS chunk pinned to segment starts so
            # (q_m, k_m) complete during pair m-1
            if not imm_next:
                # last chunk: deferred first, outprojs late (feed pair 3)
                fillers = dfr_cur + rest
            else:
                total = len(dfr_cur) + len(rest)
                fillers = []
                ri = 0
                for seg in range(4):
                    if seg < 3 and 2 * seg + 1 < len(dfr_cur):
                        fillers += [dfr_cur[2 * seg], dfr_cur[2 * seg + 1]]
                    tgt = ((seg + 1) * total + 3) // 4
                    while len(fillers) < tgt and ri < len(rest):
                        fillers.append(rest[ri])
                        ri += 1
                fillers += rest[ri:]
            dfr_cur = dfr_next
            nf = 0
            nkt = 4 * (qc + 1)  # causal: key tiles 0..nkt-1
            for hp in range(HL // 2):
                m = hp
                heads = (2 * hp, 2 * hp + 1)
                ctx_ps = {h: ps_ctx.tile([HD + 1, 512], F32, tag="ctx",
                                         name=f"ctx{h}") for h in heads}

                def emit_scores_exp(kt):
                    qs = max(0, 128 * kt - 512 * qc)  # local q start
                    sc = ps_sc.tile([128, 1024], F32, tag="sc", name="sc")
                    for i, h in enumerate(heads):
                        po = 64 * i
                        nc.tensor.matmul(
                            sc[:, 512 * i + qs:512 * (i + 1)],
                            kT[m][po:po + HD, 128 * kt:128 * (kt + 1)],
                            qT[m][po:po + HD, 512 * qc + qs:512 * (qc + 1)],
                            start=True, stop=True)
                    et = et_pool.tile([128, 1024], BF16, tag="et", name="et")
                    nc.scalar.activation(et[:, qs:1024], sc[:, qs:1024],
                                         EXP, scale=SCALE)
                    if 4 * qc <= kt < 4 * qc + 4:  # diagonal: mask k>q
                        for i in range(2):
                            nc.gpsimd.affine_select(
                                out=et[:, 512 * i + qs:512 * i + qs + 128],
                                in_=et[:, 512 * i + qs:512 * i + qs + 128],
                                compare_op=mybir.AluOpType.is_ge, fill=0.0,
                                base=0, pattern=[[1, 128]],
                                channel_multiplier=-1)
                    return et

                def emit_ctx(kt, et):
                    qs = max(0, 128 * kt - 512 * qc)
                    for i, h in enumerate(heads):
                        nc.tensor.matmul(
                            ctx_ps[h][:, qs:512],
                            v_aug[kt][:].rearrange(
                                "p (h c) -> p h c", h=HL)[:, h, :],
                            et[:, 512 * i + qs:512 * (i + 1)],
                            start=(kt == 0), stop=(kt == nkt - 1))

                pend = []
                for kt in range(nkt):
                    pend.append((kt, emit_scores_exp(kt)))
                    if len(pend) > 4:
                        emit_ctx(*pend.pop(0))
                    want = (len(fillers) * (hp * nkt + kt + 1)) // (HL // 2 * nkt)
                    while nf < want:
                        fillers[nf]()
                        nf += 1
                for p in pend:
                    emit_ctx(*p)

                # normalize: one cu tile per pair, batched reciprocal
                cu = norm_pool.tile([128, 1024], F32, tag="cu", name="cu")
                sums_sb = norm_pool.tile([1, 1024], F32, tag="sums", name="sums")
                for i, h in enumerate(heads):
                    nc.vector.tensor_copy(cu[0:HD, 512 * i:512 * (i + 1)],
                                          ctx_ps[h][0:HD, :])
                    nc.vector.tensor_copy(sums_sb[:, 512 * i:512 * (i + 1)],
                                          ctx_ps[h][HD:HD + 1, :])
                recip = norm_pool.tile([1, 1024], F32, tag="recip", name="recip")
                nc.vector.reciprocal_approx_fast(recip[:], sums_sb[:])
                rep = norm_pool.tile([HD, 1024], F32, tag="rep", name="rep")
                for i in range(2):
                    nc.gpsimd.partition_broadcast(
                        rep[:, 512 * i:512 * (i + 1)],
                        recip[:, 512 * i:512 * (i + 1)])
                for i, h in enumerate(heads):
                    po = 64 * i
                    nc.vector.tensor_mul(
                        ctxT[m][po:po + HD, 512 * qc:512 * (qc + 1)],
                        cu[0:HD, 512 * i:512 * (i + 1)],
                        rep[:, 512 * i:512 * (i + 1)])
            while nf < len(fillers):
                fillers[nf]()
                nf += 1

        for qt in range(4 * (NQC - 1), 4 * NQC):
            emit_outproj(qt)

    nc.compile()
    return nc


def _shard(inputs):
    xt = {}
    for nm in ("inputs_q", "inputs_k", "inputs_v"):
        xt[nm] = [np.ascontiguousarray(inputs[nm][b].T.astype(ml_dtypes.bfloat16))
                  for b in range(4)]
    w16 = {nm: inputs[nm].astype(ml_dtypes.bfloat16)
           for nm in ("Wq", "Wk", "Wv", "Wo")}
    in_maps = []
    for c in range(N_CORES):
        b, g = c // 2, c % 2
        sl = slice(512 * g, 512 * (g + 1))
        in_maps.append({
            "xqt": xt["inputs_q"][b],
            "xkt": xt["inputs_k"][b],
            "xvt": xt["inputs_v"][b],
            "wq": np.ascontiguousarray(w16["Wq"][:, sl]),
            "wk": np.ascontiguousarray(w16["Wk"][:, sl]),
            "wv": np.ascontiguousarray(w16["Wv"][:, sl]),
            "bq": np.ascontiguousarray(inputs["bq"][sl])[:, None],
            "wo": np.ascontiguousarray(w16["Wo"][sl, :]),
        })
    return in_maps


def kernel(**inputs):
    global _compiled
    inputs = {k: np.asarray(v, dtype=np.float32) for k, v in inputs.items()}
    if _compiled is None:
        _compiled = _build()
    nc = _compiled
    in_maps = _shard(inputs)
    res = run_bass_kernel_spmd(nc, in_maps, list(range(N_CORES)),
                               trace=bool(int(__import__("os").environ.get("BASS_TRACE", "0"))))
    kernel.last_results = res
    B = 4
    b_eff = inputs["bv"] @ inputs["Wo"] + inputs["bo"]
    out = np.empty((B, S, D), np.float32)
    for b in range(B):
        out[b] = res.results[2 * b]["out"] + res.results[2 * b + 1]["out"]
    out += b_eff[None, None, :]
    return out
